# revision 1
# baseline (speedup 1.0000x reference)
"""DGCNN forward on 8 NeuronCores — data-parallel over batch (1 point cloud/core).

Per-core pipeline per EdgeConv layer (exact fp32):
  - augmented matmul on PE -> neg-distance matrix D [1024,1024] (symmetric)
  - top-24 extraction per row via DVE MAX8/FIND_INDEX8/MATCH_REPLACE8
  - threshold t = (v20+v21)/2 -> second PE pass computes D - t, ACT Sign -> +/-1 mask
  - BN batch stats via mask-matmuls on PE + one small AllReduce per BN layer
  - neighbor-max via chunked SWDGE dma_gather (rank-major) + DVE max chain
  - BN apply + LeakyReLU(0.2) after PE transpose back to [C,N]
Head: 1x1 conv (W5) + BN + lrelu, global max/mean pool, 3 FCs with batch BNs.
"""
import numpy as np

N = 1024
K = 20
EPS = 1e-5
B = 8
NRANK = 19          # gathered ranks 1..19 (rank 0 == self == At)
NIDX = NRANK * N    # 19456
GCH = 38            # gather chunks of 512 idxs
DEBUG = False

_CACHE = {}


def _build(debug=False):
    import concourse.bass as bass
    import concourse.tile as tile
    from concourse import bacc, mybir

    F32 = mybir.dt.float32
    I16 = mybir.dt.int16
    U32 = mybir.dt.uint32
    AF = mybir.ActivationFunctionType
    AL = mybir.AluOpType
    AX = mybir.AxisListType

    LAYERS = [(3, 64), (64, 64), (64, 128), (128, 256)]

    nc = bacc.Bacc("TRN2", target_bir_lowering=False, debug=False, num_devices=8)

    x_in = nc.dram_tensor("x", [3, N], F32, kind="ExternalInput")
    wn, wd, gbc = [], [], []
    for li, (ci, co) in enumerate(LAYERS):
        wn.append(nc.dram_tensor(f"wn{li}", [ci, co], F32, kind="ExternalInput"))
        wd.append(nc.dram_tensor(f"wd{li}", [ci, co], F32, kind="ExternalInput"))
        gbc.append(nc.dram_tensor(f"gb{li}", [max(1, co // 128), 128, 2], F32,
                                  kind="ExternalInput"))
    w5a = nc.dram_tensor("w5a", [64, N], F32, kind="ExternalInput")
    w5b = nc.dram_tensor("w5b", [64, N], F32, kind="ExternalInput")
    w5c = nc.dram_tensor("w5c", [128, N], F32, kind="ExternalInput")
    w5d = nc.dram_tensor("w5d", [2, 128, N], F32, kind="ExternalInput")
    gb5 = nc.dram_tensor("gb5", [8, 128, 2], F32, kind="ExternalInput")
    wl1 = nc.dram_tensor("wl1", [16, 128, 512], F32, kind="ExternalInput")
    wl2 = nc.dram_tensor("wl2", [4, 128, 256], F32, kind="ExternalInput")
    wl3 = nc.dram_tensor("wl3", [2, 128, 40], F32, kind="ExternalInput")
    gb6 = nc.dram_tensor("gb6", [2, 512], F32, kind="ExternalInput")
    gb7 = nc.dram_tensor("gb7", [2, 256], F32, kind="ExternalInput")
    bl2_in = nc.dram_tensor("bl2", [1, 256], F32, kind="ExternalInput")
    bl3_in = nc.dram_tensor("bl3", [1, 40], F32, kind="ExternalInput")
    id_in = nc.dram_tensor("ident", [128, 128], F32, kind="ExternalInput")
    out_t = nc.dram_tensor("out", [1, 40], F32, kind="ExternalOutput")
    dbg = {}
    if debug:
        for li, (ci, co) in enumerate(LAYERS):
            dbg[f"x{li}"] = nc.dram_tensor(f"dbg_x{li}", [max(1, co // 128), 128, N],
                                           F32, kind="ExternalOutput")
        dbg["negt0"] = nc.dram_tensor("dbg_negt0", [1, N], F32, kind="ExternalOutput")
        dbg["mx0"] = nc.dram_tensor("dbg_mx0", [128, 8, 64], F32, kind="ExternalOutput")
        dbg["st0"] = nc.dram_tensor("dbg_st0", [128, 1, 2], F32, kind="ExternalOutput")
        dbg["h5"] = nc.dram_tensor("dbg_h5", [8, 128, N], F32, kind="ExternalOutput")
        dbg["z"] = nc.dram_tensor("dbg_z", [128, 16], F32, kind="ExternalOutput")
        dbg["z6"] = nc.dram_tensor("dbg_z6", [1, 512], F32, kind="ExternalOutput")

    RG = [list(range(8))]

    with tile.TileContext(nc) as tc:
        import contextlib
        ctx = contextlib.ExitStack()
        with ctx:
            const = ctx.enter_context(tc.tile_pool(name="const", bufs=1))
            big = ctx.enter_context(tc.tile_pool(name="big", bufs=1))
            med = ctx.enter_context(tc.tile_pool(name="med", bufs=1))
            sm = ctx.enter_context(tc.tile_pool(name="sm", bufs=1))
            gp = ctx.enter_context(tc.tile_pool(name="gp", bufs=5))
            lrp = ctx.enter_context(tc.tile_pool(name="lrp", bufs=2))
            wlp = ctx.enter_context(tc.tile_pool(name="wlp", bufs=4))
            w5p = ctx.enter_context(tc.tile_pool(name="w5p", bufs=2))
            psA = ctx.enter_context(tc.tile_pool(name="psA", bufs=2, space="PSUM"))
            psW = ctx.enter_context(tc.tile_pool(name="psW", bufs=1, space="PSUM"))
            psT = ctx.enter_context(tc.tile_pool(name="psT", bufs=1, space="PSUM"))
            psV = ctx.enter_context(tc.tile_pool(name="psV", bufs=1, space="PSUM"))
            dram = ctx.enter_context(tc.tile_pool(name="dram", bufs=1, space="DRAM"))

            # ------------- constants -------------
            ident = const.tile([128, 128], F32, tag="ident")
            nc.sync.dma_start(ident[:], id_in.ap())
            ones = const.tile([128, 1], F32, tag="ones")
            nc.gpsimd.memset(ones[:], 1.0)
            onesr = const.tile([1, 128], F32, tag="onesr")
            nc.gpsimd.memset(onesr[:], 1.0)
            epsc = const.tile([128, 1], F32, tag="epsc")
            nc.gpsimd.memset(epsc[:], EPS)
            wns, wds, gbs = [], [], []
            for li, (ci, co) in enumerate(LAYERS):
                t1 = const.tile([ci, co], F32, tag=f"wn{li}")
                nc.sync.dma_start(t1[:], wn[li].ap())
                t2 = const.tile([ci, co], F32, tag=f"wd{li}")
                nc.sync.dma_start(t2[:], wd[li].ap())
                mt = max(1, co // 128)
                t3 = const.tile([128, mt, 2], F32, tag=f"gb{li}")
                nc.sync.dma_start(t3[:], gbc[li].ap().rearrange("m p s -> p m s"))
                wns.append(t1); wds.append(t2); gbs.append(t3)
            gb5t = const.tile([128, 8, 2], F32, tag="gb5")
            nc.sync.dma_start(gb5t[:], gb5.ap().rearrange("m p s -> p m s"))
            wl2t = [const.tile([128, 256], F32, tag=f"wl2{j}", name=f"wl2{j}") for j in range(4)]
            for j in range(4):
                nc.sync.dma_start(wl2t[j][:], wl2.ap()[j])
            wl3t = [const.tile([128, 40], F32, tag=f"wl3{j}", name=f"wl3{j}") for j in range(2)]
            for j in range(2):
                nc.sync.dma_start(wl3t[j][:], wl3.ap()[j])
            gb6t = const.tile([2, 512], F32, tag="gb6")
            nc.sync.dma_start(gb6t[:], gb6.ap())
            gb7t = const.tile([2, 256], F32, tag="gb7")
            nc.sync.dma_start(gb7t[:], gb7.ap())
            bl2t = const.tile([1, 256], F32, tag="bl2")
            nc.sync.dma_start(bl2t[:], bl2_in.ap())
            bl3t = const.tile([1, 40], F32, tag="bl3")
            nc.sync.dma_start(bl3t[:], bl3_in.ap())

            at_hbm = dram.tile([N, 256], F32, tag="at_hbm")
            idxb8 = dram.tile([8, NIDX], I16, tag="idxb8")
            tb = dram.tile([1, N], F32, tag="tb")
            zb = dram.tile([1, 512], F32, tag="zb")
            ars = [(dram.tile([2, N], F32, tag=f"ari{i}", name=f"ari{i}"),
                    dram.tile([2, N], F32, tag=f"aro{i}", addr_space="Shared",
                              name=f"aro{i}"))
                   for i in range(7)]

            x0 = const.tile([3, N], F32, tag="x0")
            nc.sync.dma_start(x0[:], x_in.ap())

            def lrelu_into(dst_ap, src_ap, P, F, scale, bias):
                """dst = lrelu_0.2(src*scale + bias), per-partition scale/bias APs."""
                z1 = lrp.tile([128, 512], F32, tag="lrz")
                z1a = z1[0:P, 0:F]
                nc.scalar.activation(z1a, src_ap, AF.Identity, bias=bias, scale=scale)
                u = lrp.tile([128, 512], F32, tag="lru")
                ua = u[0:P, 0:F]
                nc.scalar.activation(ua, z1a, AF.Abs, scale=0.4)
                nc.vector.scalar_tensor_tensor(dst_ap, z1a, 0.6, ua,
                                               op0=AL.mult, op1=AL.add)

            # ================= EdgeConv layers =================
            xs = []
            x_cur = x0
            for li, (ci, co) in enumerate(LAYERS):
                mt = max(1, co // 128)
                xc = x_cur

                # --- squares & xx row ---
                sq = med.tile([128, 1024], F32, tag="sttjunk", name="sq")
                nc.scalar.activation(sq[0:ci, :], xc[0:ci, :], AF.Square)
                fu = ci <= 64
                xof = ci if fu else 0
                augL = med.tile([128, 1024], F32, tag="x2r", name="augL")
                nc.gpsimd.memset(augL[0:xof + 3, :], 1.0)
                augR = med.tile([128, 1024], F32, tag="augR")
                nc.gpsimd.memset(augR[0:xof + 3, :], -1.0)
                xx = sm.tile([1, N], F32, tag="a2l", name="xxr")
                for hf in range(2):
                    xxp = psV.tile([1, 512], F32, tag="psV")
                    nc.tensor.matmul(xxp[:], ones[0:ci, :],
                                     sq[0:ci, 512 * hf:512 * hf + 512],
                                     start=True, stop=True)
                    nc.scalar.copy(xx[:, 512 * hf:512 * hf + 512], xxp[:])
                xxneg = sm.tile([1, N], F32, tag="ntrow", name="xxneg")
                nc.scalar.activation(xxneg[:], xx[:], AF.Copy, scale=-1.0)

                # --- aug operands (fused for ci<=64) ---
                if fu:
                    nc.scalar.activation(augL[0:ci, :], xc[0:ci, :], AF.Copy)
                    x2r = augR
                else:
                    x2r = med.tile([128, 1024], F32, tag="S2p", name="x2rb")
                nc.scalar.activation(x2r[0:ci, :], xc[0:ci, :], AF.Copy, scale=2.0)
                nc.sync.dma_start(augL[xof:xof + 1, :], xx[:])
                nc.sync.dma_start(augR[xof + 1:xof + 2, :], xxneg[:])

                # --- distance pass 1 + selection ---
                negt = sm.tile([128, 8], F32, tag="negt")
                tv = med.tile([24, 1024], F32, tag="S1p", name="Tf")
                for ch in range(8):
                    dsc = big.tile([128, 1024], F32, tag=f"big{ch}")
                    for hf in range(2):
                        dps = psA.tile([128, 512], F32, tag="psA")
                        sl = slice(512 * hf, 512 * hf + 512)
                        if fu:
                            nc.tensor.matmul(dps[:],
                                             augL[0:ci + 2, 128 * ch:128 * ch + 128],
                                             augR[0:ci + 2, sl], start=True, stop=True)
                        else:
                            nc.tensor.matmul(dps[:], xc[0:ci, 128 * ch:128 * ch + 128],
                                             x2r[0:ci, sl], start=True, stop=False)
                            nc.tensor.matmul(dps[:],
                                             augL[0:2, 128 * ch:128 * ch + 128],
                                             augR[0:2, sl], start=False, stop=True)
                        nc.scalar.copy(dsc[:, sl], dps[:])
                    vals = sm.tile([128, 24], F32, tag=f"vals{ch}")
                    idxu = sm.tile([128, 24], U32, tag=f"idxu{ch}")
                    for r in range(3):
                        rs = slice(8 * r, 8 * r + 8)
                        nc.vector.max(vals[:, rs], dsc[:])
                        nc.vector.max_index(idxu[:, rs], vals[:, rs], dsc[:])
                        if r < 2:
                            nc.vector.match_replace(dsc[:], vals[:, rs], dsc[:], -1e30)
                    nc.vector.tensor_tensor(negt[:, ch:ch + 1], vals[:, 19:20],
                                            vals[:, 20:21], op=AL.add)
                    idxf = sm.tile([128, 24], F32, tag=f"idxf{ch}")
                    nc.vector.tensor_copy(idxf[:], idxu[:])
                    tp = psT.tile([128, 128], F32, tag="psT")
                    nc.tensor.transpose(tp[0:24, :], idxf[:], ident[:])
                    nc.scalar.copy(tv[:, 128 * ch:128 * ch + 128], tp[0:24, :])
                nc.vector.tensor_scalar_mul(negt[:], negt[:], -0.5)

                # --- negt row via transpose + DRAM bounce ---
                ntp = psT.tile([128, 128], F32, tag="psT")
                nc.tensor.transpose(ntp[0:8, :], negt[:], ident[:])
                nts = sm.tile([8, 128], F32, tag="ntsb")
                nc.scalar.copy(nts[:], ntp[0:8, :])
                nc.sync.dma_start(tb[:].rearrange("o (r c) -> (o r) c", r=8), nts[:])
                nc.sync.dma_start(augR[xof + 2:xof + 3, :], tb[:])
                if debug and li == 0:
                    nc.sync.dma_start(dbg["negt0"].ap(), augR[xof + 2:xof + 3, :])

                # --- idx int16 + wrap bounce ---
                t16 = sm.tile([20, 1024], I16, tag="t16")
                nc.vector.tensor_copy(t16[0:20, :], tv[0:20, :])
                nc.sync.dma_start(idxb8[0:1, :].rearrange("o (k n) -> (o k) n", k=19),
                                  t16[1:20, :])
                idxw = med.tile([128, NIDX // 16], I16, tag="idxw")
                src_w = idxb8[0:1, :].rearrange("o (k nh b p) -> (o p) k nh b",
                                                k=19, nh=8, b=8, p=16)
                for r_ in range(8):
                    nc.sync.dma_start(idxw[16 * r_:16 * r_ + 16, :], src_w)

                # --- At / At2 / BvT + Bv rows + rsA col ---
                At = med.tile([128, 8, co], F32, tag="At")
                BvT = med.tile([128, 8, co], F32, tag="BvT")
                for ch in range(8):
                    ap1 = psA.tile([128, co], F32, tag="psB")
                    nc.tensor.matmul(ap1[:], xc[0:ci, 128 * ch:128 * ch + 128],
                                     wns[li][:], start=True, stop=True)
                    nc.scalar.copy(At[:, ch, :], ap1[:])
                    ap2 = psA.tile([128, co], F32, tag="psB")
                    nc.tensor.matmul(ap2[:], xc[0:ci, 128 * ch:128 * ch + 128],
                                     wds[li][:], start=True, stop=True)
                    nc.scalar.copy(BvT[:, ch, :], ap2[:])
                At2 = med.tile([128, 8, co], F32, tag="At2")
                nc.scalar.activation(At2[:].rearrange("p a b -> p (a b)"),
                                     At[:].rearrange("p a b -> p (a b)"), AF.Square)
                nc.sync.dma_start(
                    at_hbm[:].rearrange("(ch p) c -> p ch c", ch=8)[:, :, 0:co], At[:])
                Bv = [med.tile([128, 1024], F32, tag=f"bvr{mi}", name=f"bvr{mi}") for mi in range(mt)]
                rows_mi = co if mt == 1 else 128
                for mi in range(mt):
                    for hf in range(2):
                        bp = psA.tile([128, 512], F32, tag="psA")
                        nc.tensor.matmul(bp[0:rows_mi, :],
                                         wds[li][:, 128 * mi:128 * mi + rows_mi],
                                         xc[0:ci, 512 * hf:512 * hf + 512],
                                         start=True, stop=True)
                        nc.scalar.copy(Bv[mi][0:rows_mi, 512 * hf:512 * hf + 512],
                                       bp[0:rows_mi, :])

                # --- gather chain issue (GpSimd; 512-idx chunks). DVE merges are
                # emitted after the stats block so stats DVE work isn't queued
                # behind gather-paced merges. AR trigger is interleaved at
                # chunk 30 inside this GpSimd stream (see below).
                def issue_gathers(lo, hi):
                    for ci2 in range(lo, hi):
                        gt = gp.tile([128, 4, co], F32, tag="gch", name=f"gt{ci2}")
                        nc.gpsimd.dma_gather(gt[:], at_hbm[:][:, 0:co],
                                             idxw[:, 32 * ci2:32 * ci2 + 32],
                                             num_idxs=512, num_idxs_reg=512,
                                             elem_size=co, elem_step=256)
                        gts.append(gt)
                gts = []
                issue_gathers(0, 30)

                # --- distance pass 2 (-t) -> Sign mask tiles ---
                MTs = []
                for ch in range(8):
                    mtile = big.tile([128, 1024], F32, tag=f"big{ch}")
                    for hf in range(2):
                        sl = slice(512 * hf, 512 * hf + 512)
                        mps = psA.tile([128, 512], F32, tag="psA")
                        if fu:
                            nc.tensor.matmul(mps[:],
                                             augL[0:ci + 3, 128 * ch:128 * ch + 128],
                                             augR[0:ci + 3, sl], start=True, stop=True)
                        else:
                            nc.tensor.matmul(mps[:], xc[0:ci, 128 * ch:128 * ch + 128],
                                             x2r[0:ci, sl], start=True, stop=False)
                            nc.tensor.matmul(mps[:],
                                             augL[0:3, 128 * ch:128 * ch + 128],
                                             augR[0:3, sl], start=False, stop=True)
                        nc.vector.tensor_scalar(mtile[:, sl], mps[:], 0.0,
                                                None, op0=AL.is_ge)
                    MTs.append(mtile)

                # --- stats: 0/1 masks. cnt[n] = row-sum of mask; per-channel
                # P1 = At.cnt, P2 = At2.cnt via 1-col chained matmuls; S1 via
                # mask-matmul only for the Bv.S1 cross term. ---
                cntc = sm.tile([128, 8], F32, tag="cntc")
                for ch in range(8):
                    nc.vector.tensor_reduce(cntc[:, ch:ch + 1], MTs[ch][:],
                                            axis=AX.X, op=AL.add)
                scol = sm.tile([128, mt, 8], F32, tag="scol")
                junk = med.tile([128, 1024], F32, tag="sttjunk")
                Pc = sm.tile([128, mt, 2], F32, tag="Pc")
                for mi in range(mt):
                    pp = psA.tile([128, 2], F32, tag="psB")
                    for ch in range(8):
                        nc.tensor.matmul(pp[0:rows_mi, 0:1],
                                         At[:, ch, 128 * mi:128 * mi + rows_mi],
                                         cntc[:, ch:ch + 1],
                                         start=(ch == 0), stop=(ch == 7))
                    for ch in range(8):
                        nc.tensor.matmul(pp[0:rows_mi, 1:2],
                                         At2[:, ch, 128 * mi:128 * mi + rows_mi],
                                         cntc[:, ch:ch + 1],
                                         start=(ch == 0), stop=(ch == 7))
                    nc.scalar.copy(Pc[0:rows_mi, mi, :], pp[0:rows_mi, :])
                    S1 = med.tile([128, 1024], F32, tag="S1p")
                    for hf in range(2):
                        sl = slice(512 * hf, 512 * hf + 512)
                        sp = psA.tile([128, 512], F32, tag="psA")
                        for ch in range(8):
                            nc.tensor.matmul(sp[0:rows_mi, :],
                                             At[:, ch, 128 * mi:128 * mi + rows_mi],
                                             MTs[ch][:, sl],
                                             start=(ch == 0), stop=(ch == 7))
                        nc.scalar.copy(S1[0:rows_mi, sl], sp[0:rows_mi, :])
                    nc.vector.tensor_reduce(scol[:, mi, 2:3], Bv[mi][:], axis=AX.X, op=AL.add)
                    nc.vector.scalar_tensor_tensor(junk[:], Bv[mi][:], 1.0, S1[:],
                                                   op0=AL.mult, op1=AL.mult,
                                                   accum_out=scol[:, mi, 3:4])
                    nc.vector.scalar_tensor_tensor(junk[:], Bv[mi][:], 1.0, Bv[mi][:],
                                                   op0=AL.mult, op1=AL.mult,
                                                   accum_out=scol[:, mi, 4:5])
                pay = sm.tile([128, mt, 2], F32, tag="pay")
                for mi in range(mt):
                    # sum_y = P1 + 20*Bvs
                    nc.vector.scalar_tensor_tensor(pay[:, mi, 0:1], scol[:, mi, 2:3],
                                                   20.0, Pc[:, mi, 0:1],
                                                   op0=AL.mult, op1=AL.add)
                    # ssq_y = P2 + 2*BvS1s + 20*Bv2s
                    nc.vector.scalar_tensor_tensor(pay[:, mi, 1:2], scol[:, mi, 3:4],
                                                   2.0, Pc[:, mi, 1:2],
                                                   op0=AL.mult, op1=AL.add)
                    nc.vector.scalar_tensor_tensor(pay[:, mi, 1:2], scol[:, mi, 4:5],
                                                   20.0, pay[:, mi, 1:2],
                                                   op0=AL.mult, op1=AL.add)
                ari, aro = ars[li]
                for s_ in range(2):
                    nc.sync.dma_start(
                        ari[s_:s_ + 1, 0:128 * mt].rearrange("o (m p) -> (o p) m", p=128),
                        pay[:, :, s_])
                nc.gpsimd.collective_compute(
                    "AllReduce", AL.add, replica_groups=RG,
                    ins=[ari[:].opt()], outs=[aro[:].opt()])
                issue_gathers(30, GCH)
                stc = sm.tile([128, mt, 2], F32, tag="stc")
                for s_ in range(2):
                    nc.sync.dma_start(
                        stc[:, :, s_],
                        aro[s_:s_ + 1, 0:128 * mt].rearrange("o (m p) -> (o p) m", p=128))
                if debug and li == 0:
                    nc.sync.dma_start(dbg["st0"].ap(), stc[:])

                # --- merge gathered ranks into MxT (DVE; drains at gather pace) ---
                MxT = med.tile([128, 8, co], F32, tag="MxT")
                nc.vector.tensor_copy(MxT[:].rearrange("p a b -> p (a b)"),
                                      At[:].rearrange("p a b -> p (a b)"))
                for ci2 in range(GCH):
                    half = ci2 % 2
                    mslc = MxT[:, 4 * half:4 * half + 4, :]
                    nc.vector.tensor_tensor(
                        mslc.rearrange("p a b -> p (a b)"),
                        mslc.rearrange("p a b -> p (a b)"),
                        gts[ci2][:].rearrange("p a b -> p (a b)"), op=AL.max)

                # --- alpha/beta ---
                cnt = float(B * N * K)
                ab = sm.tile([128, mt, 2], F32, tag="ab")
                vv = sm.tile([128, mt, 3], F32, tag="vv")
                for mi in range(mt):
                    nc.vector.tensor_scalar_mul(vv[:, mi, 0:1], stc[:, mi, 0:1], 1.0 / cnt)
                    nc.vector.tensor_scalar_mul(vv[:, mi, 1:2], stc[:, mi, 1:2], 1.0 / cnt)
                    nc.vector.scalar_tensor_tensor(vv[:, mi, 2:3], vv[:, mi, 0:1], -1.0,
                                                   vv[:, mi, 0:1], op0=AL.mult, op1=AL.mult)
                    nc.vector.tensor_tensor(vv[:, mi, 2:3], vv[:, mi, 1:2],
                                            vv[:, mi, 2:3], op=AL.add)
                    nc.scalar.activation(vv[:, mi, 2:3], vv[:, mi, 2:3], AF.Sqrt,
                                         bias=epsc[:])
                    nc.vector.reciprocal(vv[:, mi, 2:3], vv[:, mi, 2:3])
                    nc.vector.tensor_tensor(ab[:, mi, 0:1], gbs[li][:, mi, 0:1],
                                            vv[:, mi, 2:3], op=AL.mult)
                    nc.vector.scalar_tensor_tensor(ab[:, mi, 1:2], vv[:, mi, 0:1], -1.0,
                                                   ab[:, mi, 0:1], op0=AL.mult,
                                                   op1=AL.mult)
                    nc.vector.tensor_tensor(ab[:, mi, 1:2], gbs[li][:, mi, 1:2],
                                            ab[:, mi, 1:2], op=AL.add)

                # --- y = MxT + BvT ; transpose ; BN+lrelu ---
                nc.vector.tensor_tensor(MxT[:].rearrange("p a b -> p (a b)"),
                                        MxT[:].rearrange("p a b -> p (a b)"),
                                        BvT[:].rearrange("p a b -> p (a b)"), op=AL.add)
                if debug and li == 0:
                    nc.sync.dma_start(dbg["mx0"].ap(), MxT[:])
                xnext = [med.tile([128, 1024], F32, tag=f"xn{li}_{mi}", name=f"xn{li}_{mi}")
                         for mi in range(mt)]
                rows = co if mt == 1 else 128
                for ch in range(8):
                    for mi in range(mt):
                        zp = psT.tile([128, 128], F32, tag="psT")
                        nc.tensor.transpose(
                            zp[0:rows, :],
                            MxT[:, ch, 128 * mi:128 * mi + min(rows, 128)], ident[:])
                        lrelu_into(xnext[mi][0:rows, 128 * ch:128 * ch + 128],
                                   zp[0:rows, :], rows, 128,
                                   ab[0:rows, mi, 0:1], ab[0:rows, mi, 1:2])
                if debug:
                    for mi in range(mt):
                        nc.sync.dma_start(dbg[f"x{li}"].ap()[mi], xnext[mi][:])
                xs.append(xnext)
                x_cur = xnext[0]

            # ================= W5 conv + BN5 =================
            hparts = [(xs[0][0], 64), (xs[1][0], 64), (xs[2][0], 128),
                      (xs[3][0], 128), (xs[3][1], 128)]
            y5 = [big.tile([128, 1024], F32, tag=f"big{mi}", name=f"y5_{mi}") for mi in range(8)]
            sc5 = sm.tile([128, 8, 2], F32, tag="sc5")
            junk2 = med.tile([128, 1024], F32, tag="sttjunk")
            w5dr = [(w5a, 64, None), (w5b, 64, None), (w5c, 128, None),
                    (w5d, 128, 0), (w5d, 128, 1)]
            for mi in range(8):
                yp0 = psW.tile([128, 512], F32, tag="psW0")
                yp1 = psW.tile([128, 512], F32, tag="psW1")
                for pi, ((hx, rows), (ap_, rr, jj)) in enumerate(zip(hparts, w5dr)):
                    ws = w5p.tile([128, 128], F32, tag="w5sl")
                    src = ap_.ap()[jj] if jj is not None else ap_.ap()
                    nc.sync.dma_start(ws[0:rows, :], src[:, 128 * mi:128 * mi + 128])
                    nc.tensor.matmul(yp0[:], ws[0:rows, :], hx[0:rows, 0:512],
                                     start=(pi == 0), stop=(pi == 4))
                    nc.tensor.matmul(yp1[:], ws[0:rows, :], hx[0:rows, 512:1024],
                                     start=(pi == 0), stop=(pi == 4))
                nc.scalar.copy(y5[mi][:, 0:512], yp0[:])
                nc.scalar.copy(y5[mi][:, 512:1024], yp1[:])
                nc.vector.tensor_reduce(sc5[:, mi, 0:1], y5[mi][:], axis=AX.X, op=AL.add)
                nc.vector.scalar_tensor_tensor(junk2[:], y5[mi][:], 1.0, y5[mi][:],
                                               op0=AL.mult, op1=AL.mult,
                                               accum_out=sc5[:, mi, 1:2])
            ari, aro = ars[4]
            for s_ in range(2):
                nc.sync.dma_start(
                    ari[s_:s_ + 1, :].rearrange("o (m p) -> (o p) m", p=128),
                    sc5[:, :, s_])
            nc.gpsimd.collective_compute("AllReduce", AL.add, replica_groups=RG,
                                         ins=[ari[:].opt()], outs=[aro[:].opt()])
            st5 = sm.tile([128, 8, 2], F32, tag="st5")
            for s_ in range(2):
                nc.sync.dma_start(
                    st5[:, :, s_],
                    aro[s_:s_ + 1, :].rearrange("o (m p) -> (o p) m", p=128))
            ab5 = sm.tile([128, 8, 2], F32, tag="ab5")
            cnt5 = float(B * N)
            for mi in range(8):
                nc.vector.tensor_scalar_mul(st5[:, mi, 0:1], st5[:, mi, 0:1], 1.0 / cnt5)
                nc.vector.tensor_scalar_mul(st5[:, mi, 1:2], st5[:, mi, 1:2], 1.0 / cnt5)
                nc.vector.scalar_tensor_tensor(junk2[0:128, 0:1], st5[:, mi, 0:1], -1.0,
                                               st5[:, mi, 0:1], op0=AL.mult, op1=AL.mult)
                nc.vector.tensor_tensor(st5[:, mi, 1:2], st5[:, mi, 1:2],
                                        junk2[0:128, 0:1], op=AL.add)
                nc.scalar.activation(st5[:, mi, 1:2], st5[:, mi, 1:2], AF.Sqrt,
                                     bias=epsc[:])
                nc.vector.reciprocal(st5[:, mi, 1:2], st5[:, mi, 1:2])
                nc.vector.tensor_tensor(ab5[:, mi, 0:1], gb5t[:, mi, 0:1],
                                        st5[:, mi, 1:2], op=AL.mult)
                nc.vector.scalar_tensor_tensor(ab5[:, mi, 1:2], st5[:, mi, 0:1], -1.0,
                                               ab5[:, mi, 0:1], op0=AL.mult, op1=AL.mult)
                nc.vector.tensor_tensor(ab5[:, mi, 1:2], gb5t[:, mi, 1:2],
                                        ab5[:, mi, 1:2], op=AL.add)
            zcol = sm.tile([128, 16], F32, tag="zcol")
            for mi in range(8):
                h5 = big.tile([128, 1024], F32, tag=f"big{mi}")
                for hf in range(2):
                    s5 = slice(512 * hf, 512 * hf + 512)
                    lrelu_into(h5[:, s5], y5[mi][:, s5], 128, 512,
                               ab5[:, mi, 0:1], ab5[:, mi, 1:2])
                if debug:
                    nc.sync.dma_start(dbg["h5"].ap()[mi], h5[:])
                m8 = sm.tile([128, 8], F32, tag="m8")
                nc.vector.max(m8[:], h5[:])
                nc.vector.tensor_copy(zcol[:, mi:mi + 1], m8[:, 0:1])
                nc.vector.tensor_reduce(zcol[:, 8 + mi:9 + mi], h5[:], axis=AX.X,
                                        op=AL.add)
            nc.vector.tensor_scalar_mul(zcol[:, 8:16], zcol[:, 8:16], 1.0 / float(N))
            if debug:
                nc.sync.dma_start(dbg["z"].ap(), zcol[:])

            # ================= FC head =================
            z6p = psV.tile([1, 512], F32, tag="psV")
            for j in range(16):
                wt = wlp.tile([128, 512], F32, tag="wl1c")
                nc.sync.dma_start(wt[:], wl1.ap()[j])
                nc.tensor.matmul(z6p[:], zcol[:, j:j + 1], wt[:],
                                 start=(j == 0), stop=(j == 15))
            z6 = sm.tile([1, 512], F32, tag="z6")
            nc.scalar.copy(z6[:], z6p[:])
            z6sq = sm.tile([1, 512], F32, tag="z6sq")
            nc.vector.tensor_tensor(z6sq[:], z6[:], z6[:], op=AL.mult)
            ari, aro = ars[5]
            nc.sync.dma_start(ari[0:1, 0:512], z6[:])
            nc.sync.dma_start(ari[1:2, 0:512], z6sq[:])
            nc.gpsimd.collective_compute("AllReduce", AL.add, replica_groups=RG,
                                         ins=[ari[:].opt()], outs=[aro[:].opt()])

            def head_bn(z_row, aro_, gbt, width, sct):
                stz = sm.tile([1, 512], F32, tag=sct + "s")
                nc.sync.dma_start(stz[:, 0:width], aro_[0:1, 0:width])
                stq = sm.tile([1, 512], F32, tag=sct + "q")
                nc.sync.dma_start(stq[:, 0:width], aro_[1:2, 0:width])
                w = slice(0, width)
                nc.vector.tensor_scalar_mul(stz[:, w], stz[:, w], 1.0 / 8.0)
                nc.vector.tensor_scalar_mul(stq[:, w], stq[:, w], 1.0 / 8.0)
                v = sm.tile([1, 512], F32, tag=sct + "v")
                nc.vector.scalar_tensor_tensor(v[:, w], stz[:, w], -1.0, stz[:, w],
                                               op0=AL.mult, op1=AL.mult)
                nc.vector.tensor_tensor(v[:, w], stq[:, w], v[:, w], op=AL.add)
                nc.scalar.activation(v[:, w], v[:, w], AF.Sqrt, bias=epsc[0:1, :])
                nc.vector.reciprocal(v[:, w], v[:, w])
                zn = sm.tile([1, 512], F32, tag=sct + "zn")
                nc.vector.tensor_tensor(zn[:, w], z_row[:, w], stz[:, w],
                                        op=AL.subtract)
                nc.vector.tensor_tensor(zn[:, w], zn[:, w], v[:, w], op=AL.mult)
                nc.vector.tensor_tensor(zn[:, w], zn[:, w], gbt[0:1, 0:width],
                                        op=AL.mult)
                # add beta row: gbt row1 -> need same partition; bounce via DMA tile
                bt = sm.tile([1, 512], F32, tag=sct + "b")
                nc.sync.dma_start(bt[:, 0:width], gbt[1:2, 0:width])
                nc.vector.tensor_tensor(zn[:, w], zn[:, w], bt[:, 0:width], op=AL.add)
                ab_ = sm.tile([1, 512], F32, tag=sct + "a")
                nc.scalar.activation(ab_[:, w], zn[:, w], AF.Abs, scale=0.4)
                nc.vector.scalar_tensor_tensor(zn[:, w], zn[:, w], 0.6, ab_[:, w],
                                               op0=AL.mult, op1=AL.add)
                return zn

            z6n = head_bn(z6, aro, gb6t, 512, "hb")
            if debug:
                nc.sync.dma_start(dbg["z6"].ap(), z6n[0:1, 0:512])
            nc.sync.dma_start(zb[:], z6n[0:1, 0:512])
            z6c = sm.tile([128, 4], F32, tag="z6c")
            nc.sync.dma_start(z6c[:], zb[:].rearrange("o (c p) -> (o p) c", p=128))
            z7p = psV.tile([1, 256], F32, tag="psV")
            for j in range(4):
                nc.tensor.matmul(z7p[:], z6c[:, j:j + 1], wl2t[j][:],
                                 start=(j == 0), stop=False)
            nc.tensor.matmul(z7p[:], onesr[:, 0:1], bl2t[:], start=False, stop=True)
            z7 = sm.tile([1, 256], F32, tag="z7")
            nc.scalar.copy(z7[:], z7p[:])
            z7sq = sm.tile([1, 256], F32, tag="z7sq")
            nc.vector.tensor_tensor(z7sq[:], z7[:], z7[:], op=AL.mult)
            ari, aro = ars[6]
            nc.sync.dma_start(ari[0:1, 0:256], z7[:])
            nc.sync.dma_start(ari[1:2, 0:256], z7sq[:])
            nc.gpsimd.collective_compute("AllReduce", AL.add, replica_groups=RG,
                                         ins=[ari[:].opt()], outs=[aro[:].opt()])
            z7n = head_bn(z7, aro, gb7t, 256, "hb")
            nc.sync.dma_start(zb[:, 0:256], z7n[0:1, 0:256])
            z7c = sm.tile([128, 2], F32, tag="z7c")
            nc.sync.dma_start(z7c[:], zb[:, 0:256].rearrange("o (c p) -> (o p) c",
                                                             p=128))
            z8p = psV.tile([1, 40], F32, tag="psV")
            for j in range(2):
                nc.tensor.matmul(z8p[:], z7c[:, j:j + 1], wl3t[j][:],
                                 start=(j == 0), stop=False)
            nc.tensor.matmul(z8p[:], onesr[:, 0:1], bl3t[:], start=False, stop=True)
            zout = sm.tile([1, 40], F32, tag="zout")
            nc.scalar.copy(zout[:], z8p[:])
            nc.sync.dma_start(out_t.ap(), zout[:])
    nc.compile()
    return nc


def _host_inputs(inputs):
    f32 = np.float32
    W = [np.asarray(inputs[f"W{i}"], f32) for i in range(1, 6)]
    cins = [3, 64, 64, 128]
    common = {"ident": np.eye(128, dtype=f32)}
    for li in range(4):
        ci = cins[li]
        Wl = W[li]
        wn_ = Wl[:, :ci]
        wc_ = Wl[:, ci:]
        common[f"wn{li}"] = np.ascontiguousarray(wn_.T)
        common[f"wd{li}"] = np.ascontiguousarray((wc_ - wn_).T)
        co = Wl.shape[0]
        mt = max(1, co // 128)
        gb = np.zeros((mt, 128, 2), f32)
        gb[:, :, 0].reshape(-1)[:co] = np.asarray(inputs[f"g{li + 1}"], f32)
        gb[:, :, 1].reshape(-1)[:co] = np.asarray(inputs[f"b{li + 1}"], f32)
        common[f"gb{li}"] = gb
    W5T = np.ascontiguousarray(W[4].T)
    common["w5a"] = np.ascontiguousarray(W5T[0:64])
    common["w5b"] = np.ascontiguousarray(W5T[64:128])
    common["w5c"] = np.ascontiguousarray(W5T[128:256])
    common["w5d"] = np.ascontiguousarray(W5T[256:512].reshape(2, 128, 1024))
    gb5 = np.zeros((8, 128, 2), f32)
    gb5[:, :, 0].reshape(-1)[:] = np.asarray(inputs["g5"], f32)
    gb5[:, :, 1].reshape(-1)[:] = np.asarray(inputs["b5"], f32)
    common["gb5"] = gb5
    common["wl1"] = np.ascontiguousarray(
        np.asarray(inputs["Wl1"], f32).T.reshape(16, 128, 512))
    common["wl2"] = np.ascontiguousarray(
        np.asarray(inputs["Wl2"], f32).T.reshape(4, 128, 256))
    common["wl3"] = np.ascontiguousarray(
        np.asarray(inputs["Wl3"], f32).T.reshape(2, 128, 40))
    common["gb6"] = np.ascontiguousarray(np.stack([np.asarray(inputs["g6"], f32),
                                                   np.asarray(inputs["b6"], f32)]))
    common["gb7"] = np.ascontiguousarray(np.stack([np.asarray(inputs["g7"], f32),
                                                   np.asarray(inputs["b7"], f32)]))
    common["bl2"] = np.asarray(inputs["bl2"], f32).reshape(1, 256)
    common["bl3"] = np.asarray(inputs["bl3"], f32).reshape(1, 40)
    x = np.asarray(inputs["x"], f32)
    return [dict(common, x=np.ascontiguousarray(x[c])) for c in range(8)]


def run_spmd(inputs, debug=False, trace=False):
    import sys, os
    sys.path.insert(0, os.path.dirname(os.path.abspath(__file__)))
    try:
        import ntff_shim
        ntff_shim.install()
    except Exception:
        pass
    from concourse import bass_utils
    key = ("dbg" if debug else "rel")
    if key not in _CACHE:
        _CACHE[key] = _build(debug=debug)
    nc = _CACHE[key]
    in_maps = _host_inputs(inputs)
    res = bass_utils.run_bass_kernel_spmd(nc, in_maps, core_ids=list(range(8)),
                                          trace=trace)
    return res


def kernel(**inputs):
    res = run_spmd(inputs, debug=DEBUG, trace=False)
    out = np.concatenate([res.results[c]["out"] for c in range(8)], axis=0)
    return out.astype(np.float32)



# revision 33
# speedup vs baseline: 2.0075x; 2.0075x over previous
"""DGCNN forward on 8 NeuronCores — data-parallel over batch (1 point cloud/core).

Per-core pipeline per EdgeConv layer (exact fp32 — the kNN selection is
chaotic across layers, so features/distances must track the fp32 reference
trajectory):
  - augmented matmul on PE -> neg-distance matrix D [1024,1024] (fp32)
  - top-24 extraction per row via DVE MAX8/FIND_INDEX8/MATCH_REPLACE8;
    selection runs in two ch-halves so the first half's gathers overlap the
    second half's selection
  - threshold t = (v20+v21)/2 -> second PE pass computes D - t, is_ge -> 0/1
  - BN batch stats via mask-matmuls on PE + one small AllReduce per layer,
    issued before the second gather half so its latency hides behind them
  - neighbor-max via chunked SWDGE dma_gather (rank-major) + DVE max chain;
    the int16 index wrap uses contiguous descriptors (tv written in
    (p, half, nh, b) free order during PSUM evacuation)
  - BN apply + LeakyReLU(0.2) after PE transpose back to [C,N]
Head: 1x1 conv (W5) + BN + lrelu, global max/mean pool, 3 FCs with batch BNs.
"""
import numpy as np

N = 1024
K = 20
EPS = 1e-5
B = 8
NRANK = 19          # gathered ranks 1..19 (rank 0 == self == At)
NIDX = NRANK * N    # 19456
GCH = 38            # gather chunks of 512 idxs
HCH = 19            # chunks per half
DEBUG = False

_CACHE = {}


def _build(debug=False):
    import concourse.bass as bass
    import concourse.tile as tile
    from concourse import bacc, mybir

    F32 = mybir.dt.float32
    I16 = mybir.dt.int16
    U32 = mybir.dt.uint32
    AF = mybir.ActivationFunctionType
    AL = mybir.AluOpType
    AX = mybir.AxisListType

    LAYERS = [(3, 64), (64, 64), (64, 128), (128, 256)]

    nc = bacc.Bacc("TRN2", target_bir_lowering=False, debug=False, num_devices=8)

    x_in = nc.dram_tensor("x", [3, N], F32, kind="ExternalInput")
    wn, wd, gbc = [], [], []
    for li, (ci, co) in enumerate(LAYERS):
        wn.append(nc.dram_tensor(f"wn{li}", [ci, co], F32, kind="ExternalInput"))
        wd.append(nc.dram_tensor(f"wd{li}", [ci, co], F32, kind="ExternalInput"))
        gbc.append(nc.dram_tensor(f"gb{li}", [max(1, co // 128), 128, 2], F32,
                                  kind="ExternalInput"))
    w5a = nc.dram_tensor("w5a", [64, N], F32, kind="ExternalInput")
    w5b = nc.dram_tensor("w5b", [64, N], F32, kind="ExternalInput")
    w5c = nc.dram_tensor("w5c", [128, N], F32, kind="ExternalInput")
    w5d = nc.dram_tensor("w5d", [2, 128, N], F32, kind="ExternalInput")
    gb5 = nc.dram_tensor("gb5", [8, 128, 2], F32, kind="ExternalInput")
    wl1 = nc.dram_tensor("wl1", [16, 128, 512], F32, kind="ExternalInput")
    wl2 = nc.dram_tensor("wl2", [4, 128, 256], F32, kind="ExternalInput")
    wl3 = nc.dram_tensor("wl3", [2, 128, 40], F32, kind="ExternalInput")
    gb6 = nc.dram_tensor("gb6", [2, 512], F32, kind="ExternalInput")
    gb7 = nc.dram_tensor("gb7", [2, 256], F32, kind="ExternalInput")
    bl2_in = nc.dram_tensor("bl2", [1, 256], F32, kind="ExternalInput")
    bl3_in = nc.dram_tensor("bl3", [1, 40], F32, kind="ExternalInput")
    id_in = nc.dram_tensor("ident", [128, 128], F32, kind="ExternalInput")
    out_t = nc.dram_tensor("out", [1, 40], F32, kind="ExternalOutput")
    dbg = {}
    if debug:
        for li, (ci, co) in enumerate(LAYERS):
            dbg[f"x{li}"] = nc.dram_tensor(f"dbg_x{li}", [max(1, co // 128), 128, N],
                                           F32, kind="ExternalOutput")
        dbg["negt0"] = nc.dram_tensor("dbg_negt0", [1, N], F32, kind="ExternalOutput")
        dbg["mx0"] = nc.dram_tensor("dbg_mx0", [128, 8, 64], F32, kind="ExternalOutput")
        dbg["st0"] = nc.dram_tensor("dbg_st0", [128, 1, 2], F32, kind="ExternalOutput")
        dbg["h5"] = nc.dram_tensor("dbg_h5", [8, 128, N], F32, kind="ExternalOutput")
        dbg["z"] = nc.dram_tensor("dbg_z", [128, 16], F32, kind="ExternalOutput")
        dbg["z6"] = nc.dram_tensor("dbg_z6", [1, 512], F32, kind="ExternalOutput")

    RG = [list(range(8))]

    with tile.TileContext(nc) as tc:
        import contextlib
        ctx = contextlib.ExitStack()
        with ctx:
            const = ctx.enter_context(tc.tile_pool(name="const", bufs=1))
            big = ctx.enter_context(tc.tile_pool(name="big", bufs=1))
            med = ctx.enter_context(tc.tile_pool(name="med", bufs=1))
            sm = ctx.enter_context(tc.tile_pool(name="sm", bufs=1))
            gp = ctx.enter_context(tc.tile_pool(name="gp", bufs=5))
            lrp = ctx.enter_context(tc.tile_pool(name="lrp", bufs=2))
            wlp = ctx.enter_context(tc.tile_pool(name="wlp", bufs=4))
            w5p = ctx.enter_context(tc.tile_pool(name="w5p", bufs=2))
            psA = ctx.enter_context(tc.tile_pool(name="psA", bufs=2, space="PSUM"))
            psW = ctx.enter_context(tc.tile_pool(name="psW", bufs=1, space="PSUM"))
            psT = ctx.enter_context(tc.tile_pool(name="psT", bufs=1, space="PSUM"))
            psV = ctx.enter_context(tc.tile_pool(name="psV", bufs=1, space="PSUM"))
            dram = ctx.enter_context(tc.tile_pool(name="dram", bufs=1, space="DRAM"))

            # ------------- constants -------------
            ident = const.tile([128, 128], F32, tag="ident")
            nc.sync.dma_start(ident[:], id_in.ap())
            ones = const.tile([128, 1], F32, tag="ones")
            nc.gpsimd.memset(ones[:], 1.0)
            onesr = const.tile([1, 128], F32, tag="onesr")
            nc.gpsimd.memset(onesr[:], 1.0)
            epsc = const.tile([128, 1], F32, tag="epsc")
            nc.gpsimd.memset(epsc[:], EPS)
            wns, wds, gbs = [], [], []
            for li, (ci, co) in enumerate(LAYERS):
                t1 = const.tile([ci, co], F32, tag=f"wn{li}")
                nc.sync.dma_start(t1[:], wn[li].ap())
                t2 = const.tile([ci, co], F32, tag=f"wd{li}")
                nc.sync.dma_start(t2[:], wd[li].ap())
                mt = max(1, co // 128)
                t3 = const.tile([128, mt, 2], F32, tag=f"gb{li}")
                nc.sync.dma_start(t3[:], gbc[li].ap().rearrange("m p s -> p m s"))
                wns.append(t1); wds.append(t2); gbs.append(t3)
            gb5t = const.tile([128, 8, 2], F32, tag="gb5")
            nc.sync.dma_start(gb5t[:], gb5.ap().rearrange("m p s -> p m s"))
            wl2t = [const.tile([128, 256], F32, tag=f"wl2{j}", name=f"wl2{j}") for j in range(4)]
            for j in range(4):
                nc.sync.dma_start(wl2t[j][:], wl2.ap()[j])
            wl3t = [const.tile([128, 40], F32, tag=f"wl3{j}", name=f"wl3{j}") for j in range(2)]
            for j in range(2):
                nc.sync.dma_start(wl3t[j][:], wl3.ap()[j])
            gb6t = const.tile([2, 512], F32, tag="gb6")
            nc.sync.dma_start(gb6t[:], gb6.ap())
            gb7t = const.tile([2, 256], F32, tag="gb7")
            nc.sync.dma_start(gb7t[:], gb7.ap())
            bl2t = const.tile([1, 256], F32, tag="bl2")
            nc.sync.dma_start(bl2t[:], bl2_in.ap())
            bl3t = const.tile([1, 40], F32, tag="bl3")
            nc.sync.dma_start(bl3t[:], bl3_in.ap())

            at_hbm = dram.tile([N, 256], F32, tag="at_hbm")
            idxb8 = dram.tile([8, NIDX], I16, tag="idxb8")
            tb = dram.tile([1, N], F32, tag="tb")
            zb = dram.tile([1, 512], F32, tag="zb")
            ars = [(dram.tile([2, N], F32, tag=f"ari{i}", name=f"ari{i}"),
                    dram.tile([2, N], F32, tag=f"aro{i}", addr_space="Shared",
                              name=f"aro{i}"))
                   for i in range(7)]

            x0 = const.tile([3, N], F32, tag="x0")
            nc.sync.dma_start(x0[:], x_in.ap())

            def lrelu_into(dst_ap, src_ap, P, F, scale, bias):
                """dst = lrelu_0.2(src*scale + bias), per-partition scale/bias APs."""
                z1 = lrp.tile([128, 512], F32, tag="lrz")
                z1a = z1[0:P, 0:F]
                nc.scalar.activation(z1a, src_ap, AF.Identity, bias=bias, scale=scale)
                u = lrp.tile([128, 512], F32, tag="lru")
                ua = u[0:P, 0:F]
                nc.scalar.activation(ua, z1a, AF.Abs, scale=0.4)
                nc.vector.scalar_tensor_tensor(dst_ap, z1a, 0.6, ua,
                                               op0=AL.mult, op1=AL.add)

            # ================= EdgeConv layers =================
            xs = []
            x_cur = x0
            for li, (ci, co) in enumerate(LAYERS):
                mt = max(1, co // 128)
                xc = x_cur

                # --- squares & xx row ---
                sq = med.tile([128, 1024], F32, tag="sttjunk", name="sq")
                nc.scalar.activation(sq[0:ci, :], xc[0:ci, :], AF.Square)
                fu = ci <= 64
                xof = ci if fu else 0
                augL = med.tile([128, 1024], F32, tag="x2r", name="augL")
                nc.gpsimd.memset(augL[0:xof + 3, :], 1.0)
                augR = med.tile([128, 1024], F32, tag="augR")
                nc.gpsimd.memset(augR[0:xof + 3, :], -1.0)
                xx = sm.tile([1, N], F32, tag="a2l", name="xxr")
                for hf in range(2):
                    xxp = psV.tile([1, 512], F32, tag="psV")
                    nc.tensor.matmul(xxp[:], ones[0:ci, :],
                                     sq[0:ci, 512 * hf:512 * hf + 512],
                                     start=True, stop=True)
                    nc.scalar.copy(xx[:, 512 * hf:512 * hf + 512], xxp[:])
                xxneg = sm.tile([1, N], F32, tag="ntrow", name="xxneg")
                nc.scalar.activation(xxneg[:], xx[:], AF.Copy, scale=-1.0)

                # --- aug operands (fused for ci<=64) ---
                if fu:
                    nc.scalar.activation(augL[0:ci, :], xc[0:ci, :], AF.Copy)
                    x2r = augR
                else:
                    x2r = med.tile([128, 1024], F32, tag="S2p", name="x2rb")
                nc.scalar.activation(x2r[0:ci, :], xc[0:ci, :], AF.Copy, scale=2.0)
                nc.sync.dma_start(augL[xof:xof + 1, :], xx[:])
                nc.sync.dma_start(augR[xof + 1:xof + 2, :], xxneg[:])

                # --- At / At2 / BvT + Bv rows (only need xc; early so the
                # at_hbm payload is ready before the first gather half) ---
                At = med.tile([128, 8, co], F32, tag="At")
                BvT = med.tile([128, 8, co], F32, tag="BvT")
                for ch in range(8):
                    ap1 = psA.tile([128, co], F32, tag="psB")
                    nc.tensor.matmul(ap1[:], xc[0:ci, 128 * ch:128 * ch + 128],
                                     wns[li][:], start=True, stop=True)
                    nc.scalar.copy(At[:, ch, :], ap1[:])
                    ap2 = psA.tile([128, co], F32, tag="psB")
                    nc.tensor.matmul(ap2[:], xc[0:ci, 128 * ch:128 * ch + 128],
                                     wds[li][:], start=True, stop=True)
                    nc.scalar.copy(BvT[:, ch, :], ap2[:])
                At2 = med.tile([128, 8, co], F32, tag="At2")
                nc.scalar.activation(At2[:].rearrange("p a b -> p (a b)"),
                                     At[:].rearrange("p a b -> p (a b)"), AF.Square)
                nc.sync.dma_start(
                    at_hbm[:].rearrange("(ch p) c -> p ch c", ch=8)[:, :, 0:co], At[:])
                Bv = [med.tile([128, 1024], F32, tag=f"bvr{mi}", name=f"bvr{mi}") for mi in range(mt)]
                rows_mi = co if mt == 1 else 128
                for mi in range(mt):
                    for hf in range(2):
                        bp = psA.tile([128, 512], F32, tag="psA")
                        nc.tensor.matmul(bp[0:rows_mi, :],
                                         wds[li][:, 128 * mi:128 * mi + rows_mi],
                                         xc[0:ci, 512 * hf:512 * hf + 512],
                                         start=True, stop=True)
                        nc.scalar.copy(Bv[mi][0:rows_mi, 512 * hf:512 * hf + 512],
                                       bp[0:rows_mi, :])

                # --- distance pass 1 + selection, in two ch-halves ---
                negt = sm.tile([128, 8], F32, tag="negt")
                tv = med.tile([24, 1024], F32, tag="S1p", name="Tf")
                t16 = sm.tile([20, 1024], I16, tag="t16")
                idxw = med.tile([128, NIDX // 16], I16, tag="idxw")
                gts = []

                def sel_ch(ch):
                    dsc = big.tile([128, 1024], F32, tag=f"big{ch}")
                    for hf in range(2):
                        dps = psA.tile([128, 512], F32, tag="psA")
                        sl = slice(512 * hf, 512 * hf + 512)
                        if fu:
                            nc.tensor.matmul(dps[:],
                                             augL[0:ci + 2, 128 * ch:128 * ch + 128],
                                             augR[0:ci + 2, sl], start=True, stop=True)
                        else:
                            nc.tensor.matmul(dps[:], xc[0:ci, 128 * ch:128 * ch + 128],
                                             x2r[0:ci, sl], start=True, stop=False)
                            nc.tensor.matmul(dps[:],
                                             augL[0:2, 128 * ch:128 * ch + 128],
                                             augR[0:2, sl], start=False, stop=True)
                        nc.scalar.copy(dsc[:, sl], dps[:])
                    vals = sm.tile([128, 24], F32, tag=f"vals{ch}")
                    idxu = sm.tile([128, 24], U32, tag=f"idxu{ch}")
                    for r in range(3):
                        rs = slice(8 * r, 8 * r + 8)
                        nc.vector.max(vals[:, rs], dsc[:])
                        nc.vector.max_index(idxu[:, rs], vals[:, rs], dsc[:])
                        if r < 2:
                            nc.vector.match_replace(dsc[:], vals[:, rs], dsc[:], -1e30)
                    nc.vector.tensor_tensor(negt[:, ch:ch + 1], vals[:, 19:20],
                                            vals[:, 20:21], op=AL.add)
                    idxf = sm.tile([128, 24], F32, tag=f"idxf{ch}")
                    nc.vector.tensor_copy(idxf[:], idxu[:])
                    tp = psT.tile([128, 128], F32, tag="psT")
                    nc.tensor.transpose(tp[0:24, :], idxf[:], ident[:])
                    # write ranks 0..19 into tv in (p, half, nh', b) free order
                    # (n = ch*128 + b*16 + p, ch = 4*half + nh') so the idx
                    # bounce below uses contiguous descriptors.
                    nc.scalar.copy(
                        tv[0:20, :].rearrange("r (p h c b) -> r h c p b",
                                              p=16, h=2, c=4, b=8)[:, ch // 4, ch % 4],
                        tp[0:20, :].rearrange("r (b p) -> r p b", b=8, p=16))

                def wrap_half(h):
                    # tv free order (p, h, g=32); DRAM layout [p][h][k][g]
                    nc.vector.tensor_copy(
                        t16[0:20, :].rearrange("k (p h g) -> k h p g",
                                               p=16, h=2, g=32)[:, h],
                        tv[0:20, :].rearrange("r (p h g) -> r h p g",
                                              p=16, h=2, g=32)[:, h])
                    nc.sync.dma_start(
                        idxb8[0:1, :].rearrange("o (p h k g) -> (o h) k p g",
                                                p=16, h=2, k=19, g=32)[h],
                        t16[1:20, :].rearrange("k (p h g) -> k h p g",
                                               p=16, h=2, g=32)[:, h])
                    src_wh = idxb8[0:1, :].rearrange("o (p f) -> (o p) f",
                                                     p=16)[:, 608 * h:608 * h + 608]
                    for r_ in range(8):
                        nc.sync.dma_start(
                            idxw[16 * r_:16 * r_ + 16, 608 * h:608 * h + 608], src_wh)

                def issue_gathers(lo, hi):
                    for ci2 in range(lo, hi):
                        gt = gp.tile([128, 4, co], F32, tag="gch", name=f"gt{ci2}")
                        nc.gpsimd.dma_gather(gt[:], at_hbm[:][:, 0:co],
                                             idxw[:, 32 * ci2:32 * ci2 + 32],
                                             num_idxs=512, num_idxs_reg=512,
                                             elem_size=co, elem_step=256)
                        gts.append(gt)

                for ch in range(4):
                    sel_ch(ch)
                wrap_half(0)
                issue_gathers(0, HCH)
                for ch in range(4, 8):
                    sel_ch(ch)
                wrap_half(1)
                nc.vector.tensor_scalar_mul(negt[:], negt[:], -0.5)

                # --- negt row via transpose + DRAM bounce ---
                ntp = psT.tile([128, 128], F32, tag="psT")
                nc.tensor.transpose(ntp[0:8, :], negt[:], ident[:])
                nts = sm.tile([8, 128], F32, tag="ntsb")
                nc.scalar.copy(nts[:], ntp[0:8, :])
                nc.sync.dma_start(tb[:].rearrange("o (r c) -> (o r) c", r=8), nts[:])
                nc.sync.dma_start(augR[xof + 2:xof + 3, :], tb[:])
                if debug and li == 0:
                    nc.sync.dma_start(dbg["negt0"].ap(), augR[xof + 2:xof + 3, :])

                # --- distance pass 2 (-t) -> is_ge mask tiles ---
                MTs = []
                for ch in range(8):
                    mraw = big.tile([128, 1024], F32, tag=f"big{ch}")
                    mtile = mraw[:]
                    for hf in range(2):
                        sl = slice(512 * hf, 512 * hf + 512)
                        mps = psA.tile([128, 512], F32, tag="psA")
                        if fu:
                            nc.tensor.matmul(mps[:],
                                             augL[0:ci + 3, 128 * ch:128 * ch + 128],
                                             augR[0:ci + 3, sl], start=True, stop=True)
                        else:
                            nc.tensor.matmul(mps[:], xc[0:ci, 128 * ch:128 * ch + 128],
                                             x2r[0:ci, sl], start=True, stop=False)
                            nc.tensor.matmul(mps[:],
                                             augL[0:3, 128 * ch:128 * ch + 128],
                                             augR[0:3, sl], start=False, stop=True)
                        nc.vector.tensor_scalar(mtile[:, sl], mps[:], 0.0,
                                                None, op0=AL.is_ge)
                    MTs.append(mtile)

                # --- stats: 0/1 masks. cnt[n] = row-sum of mask; per-channel
                # P1 = At.cnt, P2 = At2.cnt via 1-col chained matmuls; S1 via
                # mask-matmul only for the Bv.S1 cross term. ---
                cntc = sm.tile([128, 8], F32, tag="cntc")
                for ch in range(8):
                    nc.vector.tensor_reduce(cntc[:, ch:ch + 1], MTs[ch][:],
                                            axis=AX.X, op=AL.add)
                scol = sm.tile([128, mt, 8], F32, tag="scol")
                junk = med.tile([128, 1024], F32, tag="sttjunk")
                Pc = sm.tile([128, mt, 2], F32, tag="Pc")
                for mi in range(mt):
                    pp = psA.tile([128, 2], F32, tag="psB")
                    for ch in range(8):
                        nc.tensor.matmul(pp[0:rows_mi, 0:1],
                                         At[:, ch, 128 * mi:128 * mi + rows_mi],
                                         cntc[:, ch:ch + 1],
                                         start=(ch == 0), stop=(ch == 7))
                    for ch in range(8):
                        nc.tensor.matmul(pp[0:rows_mi, 1:2],
                                         At2[:, ch, 128 * mi:128 * mi + rows_mi],
                                         cntc[:, ch:ch + 1],
                                         start=(ch == 0), stop=(ch == 7))
                    nc.scalar.copy(Pc[0:rows_mi, mi, :], pp[0:rows_mi, :])
                    S1 = med.tile([128, 1024], F32, tag="S1p")
                    for hf in range(2):
                        sl = slice(512 * hf, 512 * hf + 512)
                        sp = psA.tile([128, 512], F32, tag="psA")
                        for ch in range(8):
                            nc.tensor.matmul(sp[0:rows_mi, :],
                                             At[:, ch, 128 * mi:128 * mi + rows_mi],
                                             MTs[ch][:, sl],
                                             start=(ch == 0), stop=(ch == 7))
                        nc.scalar.copy(S1[0:rows_mi, sl], sp[0:rows_mi, :])
                    nc.vector.tensor_reduce(scol[:, mi, 2:3], Bv[mi][:], axis=AX.X, op=AL.add)
                    nc.vector.scalar_tensor_tensor(junk[:], Bv[mi][:], 1.0, S1[:],
                                                   op0=AL.mult, op1=AL.mult,
                                                   accum_out=scol[:, mi, 3:4])
                    nc.vector.scalar_tensor_tensor(junk[:], Bv[mi][:], 1.0, Bv[mi][:],
                                                   op0=AL.mult, op1=AL.mult,
                                                   accum_out=scol[:, mi, 4:5])
                pay = sm.tile([128, mt, 2], F32, tag="pay")
                for mi in range(mt):
                    # sum_y = P1 + 20*Bvs
                    nc.vector.scalar_tensor_tensor(pay[:, mi, 0:1], scol[:, mi, 2:3],
                                                   20.0, Pc[:, mi, 0:1],
                                                   op0=AL.mult, op1=AL.add)
                    # ssq_y = P2 + 2*BvS1s + 20*Bv2s
                    nc.vector.scalar_tensor_tensor(pay[:, mi, 1:2], scol[:, mi, 3:4],
                                                   2.0, Pc[:, mi, 1:2],
                                                   op0=AL.mult, op1=AL.add)
                    nc.vector.scalar_tensor_tensor(pay[:, mi, 1:2], scol[:, mi, 4:5],
                                                   20.0, pay[:, mi, 1:2],
                                                   op0=AL.mult, op1=AL.add)
                ari, aro = ars[li]
                for s_ in range(2):
                    nc.sync.dma_start(
                        ari[s_:s_ + 1, 0:128 * mt].rearrange("o (m p) -> (o p) m", p=128),
                        pay[:, :, s_])
                nc.gpsimd.collective_compute(
                    "AllReduce", AL.add, replica_groups=RG,
                    ins=[ari[:].opt()], outs=[aro[:].opt()])
                issue_gathers(HCH, GCH)
                stc = sm.tile([128, mt, 2], F32, tag="stc")
                for s_ in range(2):
                    nc.sync.dma_start(
                        stc[:, :, s_],
                        aro[s_:s_ + 1, 0:128 * mt].rearrange("o (m p) -> (o p) m", p=128))
                if debug and li == 0:
                    nc.sync.dma_start(dbg["st0"].ap(), stc[:])

                # --- merge gathered ranks into MxT (DVE; drains at gather pace) ---
                MxT = med.tile([128, 8, co], F32, tag="MxT")
                nc.vector.tensor_copy(MxT[:].rearrange("p a b -> p (a b)"),
                                      At[:].rearrange("p a b -> p (a b)"))
                for ci2 in range(GCH):
                    half = ci2 // HCH
                    mslc = MxT[:, 4 * half:4 * half + 4, :]
                    nc.vector.tensor_tensor(
                        mslc.rearrange("p a b -> p (a b)"),
                        mslc.rearrange("p a b -> p (a b)"),
                        gts[ci2][:].rearrange("p a b -> p (a b)"), op=AL.max)

                # --- alpha/beta ---
                cnt = float(B * N * K)
                ab = sm.tile([128, mt, 2], F32, tag="ab")
                vv = sm.tile([128, mt, 3], F32, tag="vv")
                for mi in range(mt):
                    nc.vector.tensor_scalar_mul(vv[:, mi, 0:1], stc[:, mi, 0:1], 1.0 / cnt)
                    nc.vector.tensor_scalar_mul(vv[:, mi, 1:2], stc[:, mi, 1:2], 1.0 / cnt)
                    nc.vector.scalar_tensor_tensor(vv[:, mi, 2:3], vv[:, mi, 0:1], -1.0,
                                                   vv[:, mi, 0:1], op0=AL.mult, op1=AL.mult)
                    nc.vector.tensor_tensor(vv[:, mi, 2:3], vv[:, mi, 1:2],
                                            vv[:, mi, 2:3], op=AL.add)
                    nc.scalar.activation(vv[:, mi, 2:3], vv[:, mi, 2:3], AF.Sqrt,
                                         bias=epsc[:])
                    nc.vector.reciprocal(vv[:, mi, 2:3], vv[:, mi, 2:3])
                    nc.vector.tensor_tensor(ab[:, mi, 0:1], gbs[li][:, mi, 0:1],
                                            vv[:, mi, 2:3], op=AL.mult)
                    nc.vector.scalar_tensor_tensor(ab[:, mi, 1:2], vv[:, mi, 0:1], -1.0,
                                                   ab[:, mi, 0:1], op0=AL.mult,
                                                   op1=AL.mult)
                    nc.vector.tensor_tensor(ab[:, mi, 1:2], gbs[li][:, mi, 1:2],
                                            ab[:, mi, 1:2], op=AL.add)

                # --- y = MxT + BvT ; transpose ; BN+lrelu ---
                nc.vector.tensor_tensor(MxT[:].rearrange("p a b -> p (a b)"),
                                        MxT[:].rearrange("p a b -> p (a b)"),
                                        BvT[:].rearrange("p a b -> p (a b)"), op=AL.add)
                if debug and li == 0:
                    nc.sync.dma_start(dbg["mx0"].ap(), MxT[:])
                xnext = [med.tile([128, 1024], F32, tag=f"xn{li}_{mi}", name=f"xn{li}_{mi}")
                         for mi in range(mt)]
                rows = co if mt == 1 else 128
                for ch in range(8):
                    for mi in range(mt):
                        zp = psT.tile([128, 128], F32, tag="psT")
                        nc.tensor.transpose(
                            zp[0:rows, :],
                            MxT[:, ch, 128 * mi:128 * mi + min(rows, 128)], ident[:])
                        lrelu_into(xnext[mi][0:rows, 128 * ch:128 * ch + 128],
                                   zp[0:rows, :], rows, 128,
                                   ab[0:rows, mi, 0:1], ab[0:rows, mi, 1:2])
                if debug:
                    for mi in range(mt):
                        nc.sync.dma_start(dbg[f"x{li}"].ap()[mi], xnext[mi][:])
                xs.append(xnext)
                x_cur = xnext[0]

            # ================= W5 conv + BN5 =================
            hparts = [(xs[0][0], 64), (xs[1][0], 64), (xs[2][0], 128),
                      (xs[3][0], 128), (xs[3][1], 128)]
            y5 = [big.tile([128, 1024], F32, tag=f"big{mi}", name=f"y5_{mi}") for mi in range(8)]
            sc5 = sm.tile([128, 8, 2], F32, tag="sc5")
            junk2 = med.tile([128, 1024], F32, tag="sttjunk")
            w5dr = [(w5a, 64, None), (w5b, 64, None), (w5c, 128, None),
                    (w5d, 128, 0), (w5d, 128, 1)]
            for mi in range(8):
                yp0 = psW.tile([128, 512], F32, tag="psW0")
                yp1 = psW.tile([128, 512], F32, tag="psW1")
                for pi, ((hx, rows), (ap_, rr, jj)) in enumerate(zip(hparts, w5dr)):
                    ws = w5p.tile([128, 128], F32, tag="w5sl")
                    src = ap_.ap()[jj] if jj is not None else ap_.ap()
                    nc.sync.dma_start(ws[0:rows, :], src[:, 128 * mi:128 * mi + 128])
                    nc.tensor.matmul(yp0[:], ws[0:rows, :], hx[0:rows, 0:512],
                                     start=(pi == 0), stop=(pi == 4))
                    nc.tensor.matmul(yp1[:], ws[0:rows, :], hx[0:rows, 512:1024],
                                     start=(pi == 0), stop=(pi == 4))
                nc.scalar.copy(y5[mi][:, 0:512], yp0[:])
                nc.scalar.copy(y5[mi][:, 512:1024], yp1[:])
                nc.vector.tensor_reduce(sc5[:, mi, 0:1], y5[mi][:], axis=AX.X, op=AL.add)
                nc.vector.scalar_tensor_tensor(junk2[:], y5[mi][:], 1.0, y5[mi][:],
                                               op0=AL.mult, op1=AL.mult,
                                               accum_out=sc5[:, mi, 1:2])
            ari, aro = ars[4]
            for s_ in range(2):
                nc.sync.dma_start(
                    ari[s_:s_ + 1, :].rearrange("o (m p) -> (o p) m", p=128),
                    sc5[:, :, s_])
            nc.gpsimd.collective_compute("AllReduce", AL.add, replica_groups=RG,
                                         ins=[ari[:].opt()], outs=[aro[:].opt()])
            st5 = sm.tile([128, 8, 2], F32, tag="st5")
            for s_ in range(2):
                nc.sync.dma_start(
                    st5[:, :, s_],
                    aro[s_:s_ + 1, :].rearrange("o (m p) -> (o p) m", p=128))
            ab5 = sm.tile([128, 8, 2], F32, tag="ab5")
            cnt5 = float(B * N)
            for mi in range(8):
                nc.vector.tensor_scalar_mul(st5[:, mi, 0:1], st5[:, mi, 0:1], 1.0 / cnt5)
                nc.vector.tensor_scalar_mul(st5[:, mi, 1:2], st5[:, mi, 1:2], 1.0 / cnt5)
                nc.vector.scalar_tensor_tensor(junk2[0:128, 0:1], st5[:, mi, 0:1], -1.0,
                                               st5[:, mi, 0:1], op0=AL.mult, op1=AL.mult)
                nc.vector.tensor_tensor(st5[:, mi, 1:2], st5[:, mi, 1:2],
                                        junk2[0:128, 0:1], op=AL.add)
                nc.scalar.activation(st5[:, mi, 1:2], st5[:, mi, 1:2], AF.Sqrt,
                                     bias=epsc[:])
                nc.vector.reciprocal(st5[:, mi, 1:2], st5[:, mi, 1:2])
                nc.vector.tensor_tensor(ab5[:, mi, 0:1], gb5t[:, mi, 0:1],
                                        st5[:, mi, 1:2], op=AL.mult)
                nc.vector.scalar_tensor_tensor(ab5[:, mi, 1:2], st5[:, mi, 0:1], -1.0,
                                               ab5[:, mi, 0:1], op0=AL.mult, op1=AL.mult)
                nc.vector.tensor_tensor(ab5[:, mi, 1:2], gb5t[:, mi, 1:2],
                                        ab5[:, mi, 1:2], op=AL.add)
            zcol = sm.tile([128, 16], F32, tag="zcol")
            # prefetch Wl1 column tiles (4-deep rotation) while h5/pool computes
            wl1t = [wlp.tile([128, 512], F32, tag="wl1c", name=f"wl1c{j}")
                    for j in range(16)]
            for mi in range(8):
                h5 = big.tile([128, 1024], F32, tag=f"big{mi}")
                for hf in range(2):
                    s5 = slice(512 * hf, 512 * hf + 512)
                    lrelu_into(h5[:, s5], y5[mi][:, s5], 128, 512,
                               ab5[:, mi, 0:1], ab5[:, mi, 1:2])
                if debug:
                    nc.sync.dma_start(dbg["h5"].ap()[mi], h5[:])
                m8 = sm.tile([128, 8], F32, tag="m8")
                nc.vector.max(m8[:], h5[:])
                nc.vector.tensor_copy(zcol[:, mi:mi + 1], m8[:, 0:1])
                nc.vector.tensor_reduce(zcol[:, 8 + mi:9 + mi], h5[:], axis=AX.X,
                                        op=AL.add)
            nc.vector.tensor_scalar_mul(zcol[:, 8:16], zcol[:, 8:16], 1.0 / float(N))
            if debug:
                nc.sync.dma_start(dbg["z"].ap(), zcol[:])

            # ================= FC head =================
            z6p = psV.tile([1, 512], F32, tag="psV")
            for j in range(16):
                nc.sync.dma_start(wl1t[j][:], wl1.ap()[j])
            for j in range(16):
                nc.tensor.matmul(z6p[:], zcol[:, j:j + 1], wl1t[j][:],
                                 start=(j == 0), stop=(j == 15))
            z6 = sm.tile([1, 512], F32, tag="z6")
            nc.scalar.copy(z6[:], z6p[:])
            z6sq = sm.tile([1, 512], F32, tag="z6sq")
            nc.vector.tensor_tensor(z6sq[:], z6[:], z6[:], op=AL.mult)
            ari, aro = ars[5]
            nc.sync.dma_start(ari[0:1, 0:512], z6[:])
            nc.sync.dma_start(ari[1:2, 0:512], z6sq[:])
            nc.gpsimd.collective_compute("AllReduce", AL.add, replica_groups=RG,
                                         ins=[ari[:].opt()], outs=[aro[:].opt()])

            def head_bn(z_row, aro_, gbt, width, sct):
                stz = sm.tile([1, 512], F32, tag=sct + "s")
                nc.sync.dma_start(stz[:, 0:width], aro_[0:1, 0:width])
                stq = sm.tile([1, 512], F32, tag=sct + "q")
                nc.sync.dma_start(stq[:, 0:width], aro_[1:2, 0:width])
                w = slice(0, width)
                nc.vector.tensor_scalar_mul(stz[:, w], stz[:, w], 1.0 / 8.0)
                nc.vector.tensor_scalar_mul(stq[:, w], stq[:, w], 1.0 / 8.0)
                v = sm.tile([1, 512], F32, tag=sct + "v")
                nc.vector.scalar_tensor_tensor(v[:, w], stz[:, w], -1.0, stz[:, w],
                                               op0=AL.mult, op1=AL.mult)
                nc.vector.tensor_tensor(v[:, w], stq[:, w], v[:, w], op=AL.add)
                nc.scalar.activation(v[:, w], v[:, w], AF.Sqrt, bias=epsc[0:1, :])
                nc.vector.reciprocal(v[:, w], v[:, w])
                zn = sm.tile([1, 512], F32, tag=sct + "zn")
                nc.vector.tensor_tensor(zn[:, w], z_row[:, w], stz[:, w],
                                        op=AL.subtract)
                nc.vector.tensor_tensor(zn[:, w], zn[:, w], v[:, w], op=AL.mult)
                nc.vector.tensor_tensor(zn[:, w], zn[:, w], gbt[0:1, 0:width],
                                        op=AL.mult)
                # add beta row: gbt row1 -> need same partition; bounce via DMA tile
                bt = sm.tile([1, 512], F32, tag=sct + "b")
                nc.sync.dma_start(bt[:, 0:width], gbt[1:2, 0:width])
                nc.vector.tensor_tensor(zn[:, w], zn[:, w], bt[:, 0:width], op=AL.add)
                ab_ = sm.tile([1, 512], F32, tag=sct + "a")
                nc.scalar.activation(ab_[:, w], zn[:, w], AF.Abs, scale=0.4)
                nc.vector.scalar_tensor_tensor(zn[:, w], zn[:, w], 0.6, ab_[:, w],
                                               op0=AL.mult, op1=AL.add)
                return zn

            z6n = head_bn(z6, aro, gb6t, 512, "hb")
            if debug:
                nc.sync.dma_start(dbg["z6"].ap(), z6n[0:1, 0:512])
            nc.sync.dma_start(zb[:], z6n[0:1, 0:512])
            z6c = sm.tile([128, 4], F32, tag="z6c")
            nc.sync.dma_start(z6c[:], zb[:].rearrange("o (c p) -> (o p) c", p=128))
            z7p = psV.tile([1, 256], F32, tag="psV")
            for j in range(4):
                nc.tensor.matmul(z7p[:], z6c[:, j:j + 1], wl2t[j][:],
                                 start=(j == 0), stop=False)
            nc.tensor.matmul(z7p[:], onesr[:, 0:1], bl2t[:], start=False, stop=True)
            z7 = sm.tile([1, 256], F32, tag="z7")
            nc.scalar.copy(z7[:], z7p[:])
            z7sq = sm.tile([1, 256], F32, tag="z7sq")
            nc.vector.tensor_tensor(z7sq[:], z7[:], z7[:], op=AL.mult)
            ari, aro = ars[6]
            nc.sync.dma_start(ari[0:1, 0:256], z7[:])
            nc.sync.dma_start(ari[1:2, 0:256], z7sq[:])
            nc.gpsimd.collective_compute("AllReduce", AL.add, replica_groups=RG,
                                         ins=[ari[:].opt()], outs=[aro[:].opt()])
            z7n = head_bn(z7, aro, gb7t, 256, "hb")
            nc.sync.dma_start(zb[:, 0:256], z7n[0:1, 0:256])
            z7c = sm.tile([128, 2], F32, tag="z7c")
            nc.sync.dma_start(z7c[:], zb[:, 0:256].rearrange("o (c p) -> (o p) c",
                                                             p=128))
            z8p = psV.tile([1, 40], F32, tag="psV")
            for j in range(2):
                nc.tensor.matmul(z8p[:], z7c[:, j:j + 1], wl3t[j][:],
                                 start=(j == 0), stop=False)
            nc.tensor.matmul(z8p[:], onesr[:, 0:1], bl3t[:], start=False, stop=True)
            zout = sm.tile([1, 40], F32, tag="zout")
            nc.scalar.copy(zout[:], z8p[:])
            nc.sync.dma_start(out_t.ap(), zout[:])
    nc.compile()
    return nc


def _host_inputs(inputs):
    f32 = np.float32
    W = [np.asarray(inputs[f"W{i}"], f32) for i in range(1, 6)]
    cins = [3, 64, 64, 128]
    common = {"ident": np.eye(128, dtype=f32)}
    for li in range(4):
        ci = cins[li]
        Wl = W[li]
        wn_ = Wl[:, :ci]
        wc_ = Wl[:, ci:]
        common[f"wn{li}"] = np.ascontiguousarray(wn_.T)
        common[f"wd{li}"] = np.ascontiguousarray((wc_ - wn_).T)
        co = Wl.shape[0]
        mt = max(1, co // 128)
        gb = np.zeros((mt, 128, 2), f32)
        gb[:, :, 0].reshape(-1)[:co] = np.asarray(inputs[f"g{li + 1}"], f32)
        gb[:, :, 1].reshape(-1)[:co] = np.asarray(inputs[f"b{li + 1}"], f32)
        common[f"gb{li}"] = gb
    W5T = np.ascontiguousarray(W[4].T)
    common["w5a"] = np.ascontiguousarray(W5T[0:64])
    common["w5b"] = np.ascontiguousarray(W5T[64:128])
    common["w5c"] = np.ascontiguousarray(W5T[128:256])
    common["w5d"] = np.ascontiguousarray(W5T[256:512].reshape(2, 128, 1024))
    gb5 = np.zeros((8, 128, 2), f32)
    gb5[:, :, 0].reshape(-1)[:] = np.asarray(inputs["g5"], f32)
    gb5[:, :, 1].reshape(-1)[:] = np.asarray(inputs["b5"], f32)
    common["gb5"] = gb5
    common["wl1"] = np.ascontiguousarray(
        np.asarray(inputs["Wl1"], f32).T.reshape(16, 128, 512))
    common["wl2"] = np.ascontiguousarray(
        np.asarray(inputs["Wl2"], f32).T.reshape(4, 128, 256))
    common["wl3"] = np.ascontiguousarray(
        np.asarray(inputs["Wl3"], f32).T.reshape(2, 128, 40))
    common["gb6"] = np.ascontiguousarray(np.stack([np.asarray(inputs["g6"], f32),
                                                   np.asarray(inputs["b6"], f32)]))
    common["gb7"] = np.ascontiguousarray(np.stack([np.asarray(inputs["g7"], f32),
                                                   np.asarray(inputs["b7"], f32)]))
    common["bl2"] = np.asarray(inputs["bl2"], f32).reshape(1, 256)
    common["bl3"] = np.asarray(inputs["bl3"], f32).reshape(1, 40)
    x = np.asarray(inputs["x"], f32)
    return [dict(common, x=np.ascontiguousarray(x[c])) for c in range(8)]


def run_spmd(inputs, debug=False, trace=False):
    import sys, os
    sys.path.insert(0, os.path.dirname(os.path.abspath(__file__)))
    try:
        import ntff_shim
        ntff_shim.install()
    except Exception:
        pass
    from concourse import bass_utils
    key = ("dbg" if debug else "rel")
    if key not in _CACHE:
        _CACHE[key] = _build(debug=debug)
    nc = _CACHE[key]
    in_maps = _host_inputs(inputs)
    res = bass_utils.run_bass_kernel_spmd(nc, in_maps, core_ids=list(range(8)),
                                          trace=trace)
    return res


def kernel(**inputs):
    res = run_spmd(inputs, debug=DEBUG, trace=False)
    out = np.concatenate([res.results[c]["out"] for c in range(8)], axis=0)
    return out.astype(np.float32)
